# revision 1
# baseline (speedup 1.0000x reference)
"""Local (sliding-window) attention kernel for TRN2, 8 NeuronCores.

Sharding: core c -> batch b=c//4, head-group hg=c%4 (4 heads of 16).
Each core computes qkv projection for its heads, banded attention, and a
partial out-projection (its heads' columns of Wo). Host sums the 4
partials per batch and adds bo.

Device algorithm (per core), all matmuls in float32r:
  qkT[512,2048]  = wqk.T @ xT          (Q rows pre-scaled by 1/sqrt(hd))
  V  [2048,256]  = xT.T @ wv           (token-major; +bias, interleaved
                                        with a ones column per head -> vaug)
  per head h, key-block j (128 keys):
    S^T[k,q]     = kT_hj.T @ qT (q-window = 384 cols: blocks j..j+2)
    P^T          = exp(S^T + band_mask)
    yT_psum[65,512] += vaug_hj.T @ P^T   (row 64 = softmax denominator)
  per (h, q-range g of 512):
    rec = 1/denom_row (ACT), bcast to [64,512] via ones-matmul,
    yT = yT_psum * rec_bcast
  out[2048,1024] = yT.T @ wo  (partial; host adds across head-groups + bo)
"""

import os
import sys

import numpy as np

if "/opt/trn_rl_repo" not in sys.path:
    sys.path.insert(0, "/opt/trn_rl_repo")

B, T, D = 2, 2048, 1024
H, W = 16, 256
HD = D // H          # 64
NCORES = 8
HPC = 4              # heads per core
FB = HPC * HD        # 256 f-columns per core
NEG = -30000.0

_STATE: dict = {}


def _build_module():
    import concourse.bacc as bacc
    import concourse.tile as tile
    from concourse import mybir

    dt = mybir.dt
    AF = mybir.ActivationFunctionType
    OP = mybir.AluOpType

    nc = bacc.Bacc(
        "TRN2",
        target_bir_lowering=False,
        debug=False,
        enable_asserts=False,
        num_devices=NCORES,
    )

    f32 = dt.float32
    f32r = dt.float32r
    xT_d = nc.dram_tensor("xT", [D, T], f32r, kind="ExternalInput").ap()
    wqk_d = nc.dram_tensor("wqk", [D, 2 * FB], f32r, kind="ExternalInput").ap()
    bqk_d = nc.dram_tensor("bqk", [128, 4], f32, kind="ExternalInput").ap()
    wv_d = nc.dram_tensor("wv", [D, FB], f32r, kind="ExternalInput").ap()
    bvb_d = nc.dram_tensor("bvb", [128, FB], f32, kind="ExternalInput").ap()
    wo_d = nc.dram_tensor("wo", [FB, D], f32r, kind="ExternalInput").ap()
    tris_d = nc.dram_tensor("tris", [128, 256], f32, kind="ExternalInput").ap()
    onesd_d = nc.dram_tensor("onesd", [128, 64], f32r, kind="ExternalInput").ap()
    out_d = nc.dram_tensor("out_p", [T, D], f32, kind="ExternalOutput").ap()

    KC = D // 128     # 8 contraction chunks
    NT = T // 128     # 16 token tiles / key blocks
    NQ = T // 512     # 4 q-ranges

    with tile.TileContext(nc) as tc:
        with (
            tc.tile_pool(name="const", bufs=1) as cpool,
            tc.tile_pool(name="work", bufs=3) as wpool,
            tc.tile_pool(name="ps", bufs=8, space="PSUM") as ppool,
        ):
            # ---- persistent SBUF ----
            xTa_sb = cpool.tile([128, KC, 1024], f32r)
            xTb_sb = cpool.tile([128, KC, 1024], f32r)
            wqk_sb = cpool.tile([128, KC, 2 * FB], f32r)
            wv_sb = cpool.tile([128, KC, FB], f32r)
            wo_sb = cpool.tile([128, 2, D], f32r)
            bqk_sb = cpool.tile([128, 4], f32)
            bvb_sb = cpool.tile([128, FB], f32)
            tris_sb = cpool.tile([128, 256], f32)
            qkT_sb = cpool.tile([128, 4, T], f32r)
            vaug_sb = cpool.tile([128, NT, HPC * (HD + 1)], f32r)
            yT_sb = cpool.tile([128, 2, T], f32r)
            onesd_sb = cpool.tile([128, 64], f32r)

            # small constants first (they gate evictions downstream), then
            # [wqk, xTa] (group 0), wv (V first half), xTb (group 1 + V
            # second half), wo (out-proj, needed last)
            nc.sync.dma_start(bqk_sb[:], bqk_d[:])
            nc.sync.dma_start(bvb_sb[:], bvb_d[:])
            nc.sync.dma_start(tris_sb[:], tris_d[:])
            nc.sync.dma_start(onesd_sb[:], onesd_d[:])
            for h in range(HPC):
                nc.sync.dma_start(
                    vaug_sb[:, :, h * 65 + 64:h * 65 + 65],
                    onesd_d[:, h * 16:(h + 1) * 16, None],
                )
            for a in range(KC):
                nc.sync.dma_start(wqk_sb[:, a, :], wqk_d[a * 128:(a + 1) * 128, :])
                nc.sync.dma_start(
                    xTa_sb[:, a, :], xT_d[a * 128:(a + 1) * 128, 0:1024]
                )
            for a in range(KC):
                nc.sync.dma_start(wv_sb[:, a, :], wv_d[a * 128:(a + 1) * 128, :])
            for a in range(KC):
                nc.sync.dma_start(
                    xTb_sb[:, a, :], xT_d[a * 128:(a + 1) * 128, 1024:2048]
                )
            for f in range(2):
                nc.sync.dma_start(wo_sb[:, f, :], wo_d[f * 128:(f + 1) * 128, :])

            # ---- qkT projection: [512, 2048] ----
            # a-outer in groups of 8 PSUM tiles so the first pass streams
            # with the xT/wqk DMA arrivals instead of serializing one
            # accumulation chain against the whole load.
            def qkT_group(grp, xh_sb):
                tiles = [(m, n) for m in range(4) for n in range(2)]
                ps_g = {
                    mn: ppool.tile([128, 512], f32, tag="ps",
                                   name=f"ps_qk{grp}_{mn[0]}_{mn[1]}")
                    for mn in tiles
                }
                for a in range(KC):
                    for (m, n) in tiles:
                        nc.tensor.matmul(
                            ps_g[(m, n)][:],
                            lhsT=wqk_sb[:, a, m * 128:(m + 1) * 128],
                            rhs=xh_sb[:, a, n * 512:(n + 1) * 512],
                            start=(a == 0),
                            stop=(a == KC - 1),
                        )
                for (m, n) in tiles:
                    nc.scalar.activation(
                        qkT_sb[:, m, (2 * grp + n) * 512:(2 * grp + n + 1) * 512],
                        ps_g[(m, n)][:],
                        AF.Identity,
                        bias=bqk_sb[:, m:m + 1],
                    )

            def v_proj(t, xh_sb):
                tl = t % 8
                ps_v = ppool.tile([128, FB], f32, tag="ps", name=f"ps_v_{t}")
                for a in range(KC):
                    nc.tensor.matmul(
                        ps_v[:],
                        lhsT=xh_sb[:, a, tl * 128:(tl + 1) * 128],
                        rhs=wv_sb[:, a, :],
                        start=(a == 0),
                        stop=(a == KC - 1),
                    )
                for h in range(HPC):
                    nc.vector.tensor_tensor(
                        out=vaug_sb[:, t, h * 65:h * 65 + 64],
                        in0=ps_v[:, h * 64:(h + 1) * 64],
                        in1=bvb_sb[:, h * 64:(h + 1) * 64],
                        op=OP.add,
                    )

            qkT_group(0, xTa_sb)
            for t in range(8):
                v_proj(t, xTa_sb)
            qkT_group(1, xTb_sb)
            for t in range(8, NT):
                v_proj(t, xTb_sb)

            # ---- attention: software-pipelined over flattened (h, j) ----
            # stage A (step idx):   S^T matmul -> exp (ACT, from PSUM)
            #                       -> zero band-complement triangles (DVE)
            # stage B (idx-DELAY):  pV matmuls; at q-range tails: normalize
            #                       (+ out-projection when last head done)
            DELAY = 4
            steps = [(h, j) for h in range(HPC) for j in range(NT)]
            pT_t = {}
            ps_y = {}

            def stage_a(idx):
                h, j = steps[idx]
                po = 64 * (h % 2)
                qwin = min(384, T - 128 * j)
                ps_s = ppool.tile([128, 384], f32, tag="ps",
                                  name=f"ps_s_{idx}")
                nc.tensor.matmul(
                    ps_s[:, :qwin],
                    lhsT=qkT_sb[po:po + 64, 2 + h // 2,
                                j * 128:(j + 1) * 128],
                    rhs=qkT_sb[po:po + 64, h // 2,
                               j * 128:j * 128 + qwin],
                    start=True,
                    stop=True,
                )
                pT = wpool.tile([128, 384], f32r, bufs=DELAY + 3,
                                name=f"pT_{idx}", tag="pT")
                nc.scalar.activation(pT[:, :qwin], ps_s[:, :qwin], AF.Exp)
                nc.vector.tensor_tensor(
                    out=pT[:, 0:128], in0=pT[:, 0:128],
                    in1=tris_sb[:, 0:128], op=OP.mult,
                )
                if qwin == 384:
                    nc.gpsimd.tensor_tensor(
                        out=pT[:, 256:384], in0=pT[:, 256:384],
                        in1=tris_sb[:, 128:256], op=OP.mult,
                    )
                pT_t[idx] = pT

            def stage_b(idx):
                h, j = steps[idx]
                po = 64 * (h % 2)
                qwin = min(384, T - 128 * j)
                pT = pT_t.pop(idx)
                for g in range((128 * j) // 512,
                               (128 * j + qwin - 1) // 512 + 1):
                    c0 = max(0, 512 * g - 128 * j)
                    c1 = min(qwin, 512 * (g + 1) - 128 * j)
                    if (h, g) not in ps_y:
                        ps_y[(h, g)] = ppool.tile(
                            [65, 512], f32, tag="ps", name=f"ps_y_{h}_{g}"
                        )
                    first = (j == max(0, 4 * g - 2))
                    last = (j == min(NT - 1, 4 * g + 3))
                    d0 = 128 * j + c0 - 512 * g
                    nc.tensor.matmul(
                        ps_y[(h, g)][:, d0:d0 + (c1 - c0)],
                        lhsT=vaug_sb[:, j, h * 65:h * 65 + 65],
                        rhs=pT[:, c0:c1],
                        start=first,
                        stop=last,
                        skip_group_check=True,
                    )
                    if not last:
                        continue
                    yps = ps_y.pop((h, g))
                    rec = wpool.tile([1, 512], f32, bufs=2,
                                     name=f"rec_{h}_{g}", tag="rec")
                    nc.vector.reciprocal(rec[:], yps[64:65, :])
                    bc_sb = wpool.tile([64, 512], f32, bufs=2,
                                       name=f"bc_{h}_{g}", tag="bc")
                    nc.gpsimd.partition_broadcast(bc_sb[:], rec[0:1, :])
                    nc.vector.tensor_tensor(
                        out=yT_sb[po:po + 64, h // 2,
                                  g * 512:(g + 1) * 512],
                        in0=yps[0:64, :],
                        in1=bc_sb[:],
                        op=OP.mult,
                    )
                    if h == HPC - 1:
                        for mt in range(4 * g, 4 * g + 4):
                            for nn in range(2):
                                ps_o = ppool.tile(
                                    [128, 512], f32, tag="ps",
                                    name=f"ps_o_{mt}_{nn}",
                                )
                                for fc in range(2):
                                    nc.tensor.matmul(
                                        ps_o[:],
                                        lhsT=yT_sb[:, fc,
                                                   mt * 128:(mt + 1) * 128],
                                        rhs=wo_sb[:, fc,
                                                  nn * 512:(nn + 1) * 512],
                                        start=(fc == 0),
                                        stop=(fc == 1),
                                    )
                                o_sb = wpool.tile(
                                    [128, 512], f32, bufs=4,
                                    name=f"o_{mt}_{nn}", tag="o_sb",
                                )
                                if (mt + nn) % 2 == 0:
                                    nc.vector.tensor_copy(
                                        out=o_sb[:], in_=ps_o[:]
                                    )
                                else:
                                    nc.scalar.copy(o_sb[:], ps_o[:])
                                nc.sync.dma_start(
                                    out_d[mt * 128:(mt + 1) * 128,
                                          nn * 512:(nn + 1) * 512],
                                    o_sb[:],
                                )

            for idx in range(len(steps) + DELAY):
                if idx < len(steps):
                    stage_a(idx)
                if idx >= DELAY:
                    stage_b(idx - DELAY)

    nc.compile()
    from concourse.bass_interp import get_hw_module

    nc.m = get_hw_module(nc.m)
    return nc


def _shard_inputs(x, Wqkv, bqkv, Wo, bo):
    x = np.asarray(x, np.float32)
    Wqkv = np.asarray(Wqkv, np.float32)
    bqkv = np.asarray(bqkv, np.float32)
    Wo = np.asarray(Wo, np.float32)

    scale = 1.0 / np.sqrt(np.float32(HD))
    c_idx = np.arange(128)[:, None]
    u_idx = np.arange(128)[None, :]
    tri0 = (u_idx >= c_idx).astype(np.float32)   # keys block j vs q block j
    tri1 = (u_idx < c_idx).astype(np.float32)    # keys block j vs q block j+2
    tris = np.concatenate([tri0, tri1], axis=1)

    in_maps = []
    for c in range(NCORES):
        b, hg = divmod(c, HPC)
        r0 = hg * FB
        Wq = Wqkv[r0:r0 + FB] * scale
        Wk = Wqkv[D + r0:D + r0 + FB]
        Wv = Wqkv[2 * D + r0:2 * D + r0 + FB]
        bq = bqkv[r0:r0 + FB] * scale
        bk = bqkv[D + r0:D + r0 + FB]
        bv = bqkv[2 * D + r0:2 * D + r0 + FB]
        in_maps.append({
            "xT": np.ascontiguousarray(x[b].T),
            "wqk": np.ascontiguousarray(np.concatenate([Wq, Wk], 0).T),
            "bqk": np.ascontiguousarray(
                np.concatenate([bq, bk]).reshape(4, 128).T),
            "wv": np.ascontiguousarray(Wv.T),
            "bvb": np.ascontiguousarray(
                np.broadcast_to(bv[None, :], (128, FB))),
            "wo": np.ascontiguousarray(Wo[:, r0:r0 + FB].T),
            "tris": tris,
            "onesd": np.ones((128, 64), np.float32),
        })
    return in_maps


def kernel(x, Wqkv, bqkv, Wo, bo):
    from concourse import bass_utils

    if "nc" not in _STATE:
        _STATE["nc"] = _build_module()
    nc = _STATE["nc"]

    in_maps = _shard_inputs(x, Wqkv, bqkv, Wo, bo)
    trace = bool(os.environ.get("TRNKERN_TRACE"))
    res = bass_utils.run_bass_kernel_spmd(
        nc,
        in_maps,
        core_ids=list(range(NCORES)),
        trace=trace,
    )
    _STATE["last"] = res

    bo = np.asarray(bo, np.float32)
    out = np.empty((B, T, D), np.float32)
    for b in range(B):
        acc = res.results[b * HPC]["out_p"].astype(np.float32)
        for hg in range(1, HPC):
            acc = acc + res.results[b * HPC + hg]["out_p"]
        out[b] = acc + bo[None, :]
    return out



# revision 10
# speedup vs baseline: 1.0409x; 1.0409x over previous
"""Local (sliding-window) attention kernel for TRN2, 8 NeuronCores.

Sharding: core c -> batch b=c//4, head-group hg=c%4 (4 heads of 16).
Each core computes qkv projection for its heads, banded attention, and a
partial out-projection (its heads' columns of Wo). Host sums the 4
partials per batch and adds bo.

v2: all matmul operands bf16 (2x moving-operand stream rate vs fp32r,
keeps the PE HAM clock-gate warm), softmax normalization via
reciprocal_approx_fast + rank-1 ones-matmul partition broadcast
(replaces the 4us vector.reciprocal + 1.2us gpsimd broadcast serial
chain), bf16 output DMA (host upcasts + sums partials).

Device algorithm (per core):
  qkT[512,2048]  = wqk.T @ xT          (Q rows pre-scaled by 1/sqrt(hd))
  V  [2048,260]  = xT.T @ wv           (token-major; +bias, with a ones
                                        column per head -> vaug)
  per head h, key-block j (128 keys):
    S^T[k,q]     = kT_hj.T @ qT (q-window = 384 cols: blocks j..j+2)
    P^T          = exp(S^T) (bf16), zero band-complement triangles
    yT_psum[65,512] += vaug_hj.T @ P^T   (row 64 = softmax denominator)
  per (h, q-range g of 512):
    rec[1,512] = approx 1/denom (DVE custom op, from PSUM)
    bc[64,512] = ones[1,64].T @ rec      (PE rank-1 broadcast)
    yT (bf16)  = yT_psum * bc
  out[2048,1024] = yT.T @ wo  (partial; host adds across head-groups + bo)
"""

import os
import sys

import numpy as np

if "/opt/trn_rl_repo" not in sys.path:
    sys.path.insert(0, "/opt/trn_rl_repo")

B, T, D = 2, 2048, 1024
H, W = 16, 256
HD = D // H          # 64
NCORES = 8
HPC = 4              # heads per core
FB = HPC * HD        # 256 f-columns per core

_STATE: dict = {}


def _build_module():
    import concourse.bacc as bacc
    import concourse.tile as tile
    from concourse import mybir

    dt = mybir.dt
    AF = mybir.ActivationFunctionType
    OP = mybir.AluOpType

    nc = bacc.Bacc(
        "TRN2",
        target_bir_lowering=False,
        debug=False,
        enable_asserts=False,
        num_devices=NCORES,
    )

    f32 = dt.float32
    f32r = dt.float32r
    bf16 = dt.bfloat16
    xT_d = nc.dram_tensor("xT", [D, T], bf16, kind="ExternalInput").ap()
    wqk_d = nc.dram_tensor("wqk", [D, 2 * FB], bf16, kind="ExternalInput").ap()
    bqk_d = nc.dram_tensor("bqk", [128, 4], f32, kind="ExternalInput").ap()
    wv_d = nc.dram_tensor("wv", [D, FB], bf16, kind="ExternalInput").ap()
    bvb_d = nc.dram_tensor("bvb", [128, HPC, HD], f32, kind="ExternalInput").ap()
    wo_d = nc.dram_tensor("wo", [FB, D], bf16, kind="ExternalInput").ap()
    tris_d = nc.dram_tensor("tris", [128, 256], bf16, kind="ExternalInput").ap()
    vone_d = nc.dram_tensor("vone", [128, 64], bf16, kind="ExternalInput").ap()
    out_d = nc.dram_tensor("out_p", [T, D], bf16, kind="ExternalOutput").ap()

    KC = D // 128     # 8 contraction chunks
    NT = T // 128     # 16 token tiles / key blocks
    NQ = T // 512     # 4 q-ranges

    with tile.TileContext(nc) as tc:
        with (
            tc.tile_pool(name="const", bufs=1) as cpool,
            tc.tile_pool(name="work", bufs=3) as wpool,
            tc.tile_pool(name="ps", bufs=8, space="PSUM") as ppool,
        ):
            # ---- persistent SBUF ----
            xTa_sb = cpool.tile([128, KC, 1024], bf16)
            xTb_sb = cpool.tile([128, KC, 1024], bf16)
            wqk_sb = cpool.tile([128, KC, 2 * FB], bf16)
            wv_sb = cpool.tile([128, KC, FB], bf16)
            wo_sb = cpool.tile([128, 2, D], bf16)
            bqk_sb = cpool.tile([128, 4], f32)
            bvb_sb = cpool.tile([128, HPC, HD], f32)
            tris_sb = cpool.tile([128, 256], bf16)
            qkT_sb = cpool.tile([128, 4, T], bf16)
            vaug_sb = cpool.tile([128, NT, HPC, HD + 1], bf16)
            yTn_sb = cpool.tile([128, 2, T], bf16)

            # small constants first (they gate evictions downstream), then
            # [wqk, xTa] (group 0), wv (V first half), xTb (group 1 + V
            # second half), wo (out-proj, needed last)
            nc.sync.dma_start(bqk_sb[:], bqk_d[:])
            nc.sync.dma_start(bvb_sb[:], bvb_d[:])
            nc.sync.dma_start(tris_sb[:], tris_d[:])
            for h in range(HPC):
                nc.sync.dma_start(
                    vaug_sb[:, :, h, HD:HD + 1],
                    vone_d[:, h * 16:(h + 1) * 16, None],
                )
            for a in range(KC):
                nc.sync.dma_start(wqk_sb[:, a, :], wqk_d[a * 128:(a + 1) * 128, :])
                nc.sync.dma_start(
                    xTa_sb[:, a, :], xT_d[a * 128:(a + 1) * 128, 0:1024]
                )
            for a in range(KC):
                nc.sync.dma_start(wv_sb[:, a, :], wv_d[a * 128:(a + 1) * 128, :])
            for a in range(KC):
                nc.sync.dma_start(
                    xTb_sb[:, a, :], xT_d[a * 128:(a + 1) * 128, 1024:2048]
                )
            for f in range(2):
                nc.sync.dma_start(wo_sb[:, f, :], wo_d[f * 128:(f + 1) * 128, :])

            # ---- qkT projection: [512, 2048] ----
            # a-outer in groups of 8 PSUM tiles so the first pass streams
            # with the xT/wqk DMA arrivals instead of serializing one
            # accumulation chain against the whole load.
            def qkT_group(grp, xh_sb):
                tiles = [(m, n) for m in range(4) for n in range(2)]
                ps_g = {
                    mn: ppool.tile([128, 512], f32, tag="ps",
                                   name=f"ps_qk{grp}_{mn[0]}_{mn[1]}")
                    for mn in tiles
                }
                for a in range(KC):
                    for (m, n) in tiles:
                        nc.tensor.matmul(
                            ps_g[(m, n)][:],
                            lhsT=wqk_sb[:, a, m * 128:(m + 1) * 128],
                            rhs=xh_sb[:, a, n * 512:(n + 1) * 512],
                            start=(a == 0),
                            stop=(a == KC - 1),
                        )
                for (m, n) in tiles:
                    nc.scalar.activation(
                        qkT_sb[:, m, (2 * grp + n) * 512:(2 * grp + n + 1) * 512],
                        ps_g[(m, n)][:],
                        AF.Identity,
                        bias=bqk_sb[:, m:m + 1],
                    )

            def v_proj(t, xh_sb):
                tl = t % 8
                ps_v = ppool.tile([128, HPC, HD], f32, tag="ps", name=f"ps_v_{t}")
                for a in range(KC):
                    nc.tensor.matmul(
                        ps_v[:],
                        lhsT=xh_sb[:, a, tl * 128:(tl + 1) * 128],
                        rhs=wv_sb[:, a, :],
                        start=(a == 0),
                        stop=(a == KC - 1),
                    )
                for h in range(HPC):
                    nc.vector.tensor_tensor(
                        out=vaug_sb[:, t, h, 0:HD],
                        in0=ps_v[:, h, :],
                        in1=bvb_sb[:, h, :],
                        op=OP.add,
                    )

            qkT_group(0, xTa_sb)
            for t in range(8):
                v_proj(t, xTa_sb)
            qkT_group(1, xTb_sb)
            for t in range(8, NT):
                v_proj(t, xTb_sb)

            # ---- attention: software-pipelined over flattened (h, j) ----
            # stage A (step idx):   S^T matmul -> exp (ACT, PSUM -> bf16)
            #                       -> zero band-complement triangles
            # stage B (idx-DELAY):  pV matmuls; at q-range tails: approx
            #                       reciprocal + PE rank-1 broadcast +
            #                       normalize (+ out-projection when last
            #                       head done)
            DELAY = 4
            steps = [(h, j) for h in range(HPC) for j in range(NT)]
            pT_t = {}
            ps_y = {}

            def stage_a(idx):
                h, j = steps[idx]
                po = 64 * (h % 2)
                qwin = min(384, T - 128 * j)
                ps_s = ppool.tile([128, 384], f32, tag="ps",
                                  name=f"ps_s_{idx}")
                nc.tensor.matmul(
                    ps_s[:, :qwin],
                    lhsT=qkT_sb[po:po + 64, 2 + h // 2,
                                j * 128:(j + 1) * 128],
                    rhs=qkT_sb[po:po + 64, h // 2,
                               j * 128:j * 128 + qwin],
                    start=True,
                    stop=True,
                )
                pT = wpool.tile([128, 384], bf16, bufs=DELAY + 3,
                                name=f"pT_{idx}", tag="pT")
                nc.scalar.activation(pT[:, :qwin], ps_s[:, :qwin], AF.Exp)
                nc.vector.tensor_tensor(
                    out=pT[:, 0:128], in0=pT[:, 0:128],
                    in1=tris_sb[:, 0:128], op=OP.mult,
                )
                if qwin == 384:
                    nc.gpsimd.tensor_tensor(
                        out=pT[:, 256:384], in0=pT[:, 256:384],
                        in1=tris_sb[:, 128:256], op=OP.mult,
                    )
                pT_t[idx] = pT

            def stage_b(idx):
                h, j = steps[idx]
                po = 64 * (h % 2)
                qwin = min(384, T - 128 * j)
                pT = pT_t.pop(idx)
                for g in range((128 * j) // 512,
                               (128 * j + qwin - 1) // 512 + 1):
                    c0 = max(0, 512 * g - 128 * j)
                    c1 = min(qwin, 512 * (g + 1) - 128 * j)
                    if (h, g) not in ps_y:
                        ps_y[(h, g)] = ppool.tile(
                            [65, 512], f32, tag="ps", name=f"ps_y_{h}_{g}"
                        )
                    first = (j == max(0, 4 * g - 2))
                    last = (j == min(NT - 1, 4 * g + 3))
                    d0 = 128 * j + c0 - 512 * g
                    nc.tensor.matmul(
                        ps_y[(h, g)][:, d0:d0 + (c1 - c0)],
                        lhsT=vaug_sb[:, j, h, :],
                        rhs=pT[:, c0:c1],
                        start=first,
                        stop=last,
                        skip_group_check=True,
                    )
                    if not last:
                        continue
                    yps = ps_y.pop((h, g))
                    # reciprocal_approx_fast's custom-DVE constants misalign
                    # at a non-zero partition base: stage the denominator row
                    # at partition 0 first.
                    dn = wpool.tile([1, 512], f32, bufs=3,
                                    name=f"dn_{h}_{g}", tag="dn")
                    nc.scalar.copy(dn[:], yps[64:65, :])
                    rec = wpool.tile([1, 512], f32, bufs=3,
                                     name=f"rec_{h}_{g}", tag="rec")
                    nc.vector.reciprocal_approx_fast(rec[:], dn[:])
                    bc_sb = wpool.tile([64, 512], f32, bufs=2,
                                       name=f"bc_{h}_{g}", tag="bc")
                    nc.gpsimd.partition_broadcast(bc_sb[:], rec[0:1, :])
                    nc.vector.tensor_tensor(
                        out=yTn_sb[po:po + 64, h // 2,
                                   g * 512:(g + 1) * 512],
                        in0=yps[0:64, :],
                        in1=bc_sb[:],
                        op=OP.mult,
                    )
                    if h == HPC - 1:
                        for mt in range(4 * g, 4 * g + 4):
                            for nn in range(2):
                                ps_o = ppool.tile(
                                    [128, 512], f32, tag="ps",
                                    name=f"ps_o_{mt}_{nn}",
                                )
                                for fc in range(2):
                                    nc.tensor.matmul(
                                        ps_o[:],
                                        lhsT=yTn_sb[:, fc,
                                                    mt * 128:(mt + 1) * 128],
                                        rhs=wo_sb[:, fc,
                                                  nn * 512:(nn + 1) * 512],
                                        start=(fc == 0),
                                        stop=(fc == 1),
                                    )
                                o_sb = wpool.tile(
                                    [128, 512], bf16, bufs=4,
                                    name=f"o_{mt}_{nn}", tag="o_sb",
                                )
                                if (mt + nn) % 2 == 0:
                                    nc.vector.tensor_copy(
                                        out=o_sb[:], in_=ps_o[:]
                                    )
                                else:
                                    nc.scalar.copy(o_sb[:], ps_o[:])
                                nc.sync.dma_start(
                                    out_d[mt * 128:(mt + 1) * 128,
                                          nn * 512:(nn + 1) * 512],
                                    o_sb[:],
                                )

            for idx in range(len(steps) + DELAY):
                if idx < len(steps):
                    stage_a(idx)
                if idx >= DELAY:
                    stage_b(idx - DELAY)

    nc.compile()
    from concourse.bass_interp import get_hw_module

    nc.m = get_hw_module(nc.m)
    return nc


def _shard_inputs(x, Wqkv, bqkv, Wo, bo):
    import ml_dtypes

    bfdt = ml_dtypes.bfloat16

    x = np.asarray(x, np.float32)
    Wqkv = np.asarray(Wqkv, np.float32)
    bqkv = np.asarray(bqkv, np.float32)
    Wo = np.asarray(Wo, np.float32)

    scale = 1.0 / np.sqrt(np.float32(HD))
    c_idx = np.arange(128)[:, None]
    u_idx = np.arange(128)[None, :]
    tri0 = (u_idx >= c_idx).astype(np.float32)   # keys block j vs q block j
    tri1 = (u_idx < c_idx).astype(np.float32)    # keys block j vs q block j+2
    tris = np.concatenate([tri0, tri1], axis=1)

    in_maps = []
    for c in range(NCORES):
        b, hg = divmod(c, HPC)
        r0 = hg * FB
        Wq = Wqkv[r0:r0 + FB] * scale
        Wk = Wqkv[D + r0:D + r0 + FB]
        Wv = Wqkv[2 * D + r0:2 * D + r0 + FB]
        bq = bqkv[r0:r0 + FB] * scale
        bk = bqkv[D + r0:D + r0 + FB]
        bv = bqkv[2 * D + r0:2 * D + r0 + FB]
        in_maps.append({
            "xT": np.ascontiguousarray(x[b].T).astype(bfdt),
            "wqk": np.ascontiguousarray(
                np.concatenate([Wq, Wk], 0).T).astype(bfdt),
            "bqk": np.ascontiguousarray(
                np.concatenate([bq, bk]).reshape(4, 128).T),
            "wv": np.ascontiguousarray(Wv.T).astype(bfdt),
            "bvb": np.ascontiguousarray(
                np.broadcast_to(bv[None, :], (128, FB))
            ).reshape(128, HPC, HD),
            "wo": np.ascontiguousarray(Wo[:, r0:r0 + FB].T).astype(bfdt),
            "tris": tris.astype(bfdt),
            "vone": np.ones((128, 64), bfdt),
        })
    return in_maps


def kernel(x, Wqkv, bqkv, Wo, bo):
    from concourse import bass_utils

    if "nc" not in _STATE:
        _STATE["nc"] = _build_module()
    nc = _STATE["nc"]

    in_maps = _shard_inputs(x, Wqkv, bqkv, Wo, bo)
    trace = bool(os.environ.get("TRNKERN_TRACE"))
    res = bass_utils.run_bass_kernel_spmd(
        nc,
        in_maps,
        core_ids=list(range(NCORES)),
        trace=trace,
    )
    _STATE["last"] = res

    bo = np.asarray(bo, np.float32)
    out = np.empty((B, T, D), np.float32)
    for b in range(B):
        acc = res.results[b * HPC]["out_p"].astype(np.float32)
        for hg in range(1, HPC):
            acc = acc + res.results[b * HPC + hg]["out_p"].astype(np.float32)
        out[b] = acc + bo[None, :]
    return out


# revision 12
# speedup vs baseline: 1.2494x; 1.2003x over previous
"""Local (sliding-window) attention kernel for TRN2, 8 NeuronCores.

Sharding: core c -> batch b=c//4, head-group hg=c%4 (4 heads of 16).
Each core computes qkv projection for its heads, banded attention, and a
partial out-projection (its heads' columns of Wo). Host sums the 4
partials per batch and adds bo.

v2: all matmul operands bf16 (2x moving-operand stream rate vs fp32r,
keeps the PE HAM clock-gate warm), softmax normalization via
reciprocal_approx_fast + rank-1 ones-matmul partition broadcast
(replaces the 4us vector.reciprocal + 1.2us gpsimd broadcast serial
chain), bf16 output DMA (host upcasts + sums partials).

Device algorithm (per core):
  qkT[512,2048]  = wqk.T @ xT          (Q rows pre-scaled by 1/sqrt(hd))
  V  [2048,260]  = xT.T @ wv           (token-major; +bias, with a ones
                                        column per head -> vaug)
  per head h, key-block j (128 keys):
    S^T[k,q]     = kT_hj.T @ qT (q-window = 384 cols: blocks j..j+2)
    P^T          = exp(S^T) (bf16), zero band-complement triangles
    yT_psum[65,512] += vaug_hj.T @ P^T   (row 64 = softmax denominator)
  per (h, q-range g of 512):
    rec[1,512] = approx 1/denom (DVE custom op, from PSUM)
    bc[64,512] = ones[1,64].T @ rec      (PE rank-1 broadcast)
    yT (bf16)  = yT_psum * bc
  out[2048,1024] = yT.T @ wo  (partial; host adds across head-groups + bo)
"""

import os
import sys

import numpy as np

if "/opt/trn_rl_repo" not in sys.path:
    sys.path.insert(0, "/opt/trn_rl_repo")

B, T, D = 2, 2048, 1024
H, W = 16, 256
HD = D // H          # 64
NCORES = 8
HPC = 4              # heads per core
FB = HPC * HD        # 256 f-columns per core

_STATE: dict = {}


def _build_module():
    import concourse.bacc as bacc
    import concourse.tile as tile
    from concourse import mybir

    dt = mybir.dt
    AF = mybir.ActivationFunctionType
    OP = mybir.AluOpType

    nc = bacc.Bacc(
        "TRN2",
        target_bir_lowering=False,
        debug=False,
        enable_asserts=False,
        num_devices=NCORES,
    )

    f32 = dt.float32
    f32r = dt.float32r
    bf16 = dt.bfloat16
    xT_d = nc.dram_tensor("xT", [D, T], bf16, kind="ExternalInput").ap()
    wqk_d = nc.dram_tensor("wqk", [D, 2 * FB], bf16, kind="ExternalInput").ap()
    bqk_d = nc.dram_tensor("bqk", [128, 4], f32, kind="ExternalInput").ap()
    wv_d = nc.dram_tensor("wv", [D, FB], bf16, kind="ExternalInput").ap()
    bvb_d = nc.dram_tensor("bvb", [128, HPC, HD], f32, kind="ExternalInput").ap()
    wo_d = nc.dram_tensor("wo", [FB, D], bf16, kind="ExternalInput").ap()
    tris_d = nc.dram_tensor("tris", [128, 256], bf16, kind="ExternalInput").ap()
    vone_d = nc.dram_tensor("vone", [128, 64], bf16, kind="ExternalInput").ap()
    out_d = nc.dram_tensor("out_p", [T, D], bf16, kind="ExternalOutput").ap()

    KC = D // 128     # 8 contraction chunks
    NT = T // 128     # 16 token tiles / key blocks
    NQ = T // 512     # 4 q-ranges

    with tile.TileContext(nc) as tc:
        with (
            tc.tile_pool(name="const", bufs=1) as cpool,
            tc.tile_pool(name="work", bufs=3) as wpool,
            tc.tile_pool(name="ps", bufs=8, space="PSUM") as ppool,
        ):
            # ---- persistent SBUF ----
            xTa_sb = cpool.tile([128, KC, 1024], bf16)
            xTb_sb = cpool.tile([128, KC, 1024], bf16)
            wqk_sb = cpool.tile([128, KC, 2 * FB], bf16)
            wv_sb = cpool.tile([128, KC, FB], bf16)
            wo_sb = cpool.tile([128, 2, D], bf16)
            bqk_sb = cpool.tile([128, 4], f32)
            bvb_sb = cpool.tile([128, HPC, HD], f32)
            tris_sb = cpool.tile([128, 256], bf16)
            qkT_sb = cpool.tile([128, 4, T], bf16)
            vaug_sb = cpool.tile([128, NT, HPC, HD + 1], bf16)
            yTn_sb = cpool.tile([128, 2, T], bf16)

            # small constants first (they gate evictions downstream), then
            # [wqk, xTa] (group 0), wv (V first half), xTb (group 1 + V
            # second half), wo (out-proj, needed last)
            nc.sync.dma_start(bqk_sb[:], bqk_d[:])
            nc.sync.dma_start(bvb_sb[:], bvb_d[:])
            nc.sync.dma_start(tris_sb[:], tris_d[:])
            for h in range(HPC):
                nc.sync.dma_start(
                    vaug_sb[:, :, h, HD:HD + 1],
                    vone_d[:, h * 16:(h + 1) * 16, None],
                )
            for a in range(KC):
                nc.sync.dma_start(wqk_sb[:, a, :], wqk_d[a * 128:(a + 1) * 128, :])
                nc.sync.dma_start(
                    xTa_sb[:, a, :], xT_d[a * 128:(a + 1) * 128, 0:1024]
                )
            for a in range(KC):
                nc.sync.dma_start(wv_sb[:, a, :], wv_d[a * 128:(a + 1) * 128, :])
            for a in range(KC):
                nc.sync.dma_start(
                    xTb_sb[:, a, :], xT_d[a * 128:(a + 1) * 128, 1024:2048]
                )
            for f in range(2):
                nc.sync.dma_start(wo_sb[:, f, :], wo_d[f * 128:(f + 1) * 128, :])

            # ---- qkT projection: [512, 2048] ----
            # a-outer in groups of 8 PSUM tiles so the first pass streams
            # with the xT/wqk DMA arrivals instead of serializing one
            # accumulation chain against the whole load.
            def qkT_group(grp, xh_sb):
                tiles = [(m, n) for m in range(4) for n in range(2)]
                ps_g = {
                    mn: ppool.tile([128, 512], f32, tag="ps",
                                   name=f"ps_qk{grp}_{mn[0]}_{mn[1]}")
                    for mn in tiles
                }
                for a in range(KC):
                    for (m, n) in tiles:
                        nc.tensor.matmul(
                            ps_g[(m, n)][:],
                            lhsT=wqk_sb[:, a, m * 128:(m + 1) * 128],
                            rhs=xh_sb[:, a, n * 512:(n + 1) * 512],
                            start=(a == 0),
                            stop=(a == KC - 1),
                        )
                for (m, n) in tiles:
                    nc.scalar.activation(
                        qkT_sb[:, m, (2 * grp + n) * 512:(2 * grp + n + 1) * 512],
                        ps_g[(m, n)][:],
                        AF.Identity,
                        bias=bqk_sb[:, m:m + 1],
                    )

            def v_proj(t, xh_sb):
                tl = t % 8
                ps_v = ppool.tile([128, HPC, HD], f32, tag="ps", name=f"ps_v_{t}")
                for a in range(KC):
                    nc.tensor.matmul(
                        ps_v[:],
                        lhsT=xh_sb[:, a, tl * 128:(tl + 1) * 128],
                        rhs=wv_sb[:, a, :],
                        start=(a == 0),
                        stop=(a == KC - 1),
                    )
                for h in range(HPC):
                    nc.vector.tensor_tensor(
                        out=vaug_sb[:, t, h, 0:HD],
                        in0=ps_v[:, h, :],
                        in1=bvb_sb[:, h, :],
                        op=OP.add,
                    )

            qkT_group(0, xTa_sb)
            for t in range(8):
                v_proj(t, xTa_sb)
            qkT_group(1, xTb_sb)
            for t in range(8, NT):
                v_proj(t, xTb_sb)

            # ---- attention: software-pipelined over flattened (h, j) ----
            # stage A (step idx):   S^T matmul -> exp (ACT, PSUM -> bf16)
            #                       -> zero band-complement triangles
            # stage B (idx-DELAY):  pV matmuls
            # The (h,g)-tail normalize chain (denom copy -> approx recip ->
            # partition broadcast -> multiply) is staggered across LATER
            # python steps via `post`, so each op's inputs are already
            # complete when it reaches its engine's strict-FIFO head —
            # otherwise the chain head-blocks the DVE/GpSimd queues that
            # the per-step mask ops need, stalling the PE ~13us per group.
            DELAY = 4
            steps = [(h, j) for h in range(HPC) for j in range(NT)]
            pT_t = {}
            ps_y = {}
            post = {}

            def at_step(s, fn):
                post.setdefault(s, []).append(fn)

            def out_proj(g):
                for mt in range(4 * g, 4 * g + 4):
                    for nn in range(2):
                        ps_o = ppool.tile(
                            [128, 512], f32, tag="ps",
                            name=f"ps_o_{mt}_{nn}",
                        )
                        for fc in range(2):
                            nc.tensor.matmul(
                                ps_o[:],
                                lhsT=yTn_sb[:, fc, mt * 128:(mt + 1) * 128],
                                rhs=wo_sb[:, fc, nn * 512:(nn + 1) * 512],
                                start=(fc == 0),
                                stop=(fc == 1),
                            )
                        o_sb = wpool.tile(
                            [128, 512], bf16, bufs=4,
                            name=f"o_{mt}_{nn}", tag="o_sb",
                        )
                        if (mt + nn) % 2 == 0:
                            nc.vector.tensor_copy(out=o_sb[:], in_=ps_o[:])
                        else:
                            nc.scalar.copy(o_sb[:], ps_o[:])
                        nc.sync.dma_start(
                            out_d[mt * 128:(mt + 1) * 128,
                                  nn * 512:(nn + 1) * 512],
                            o_sb[:],
                        )

            def stage_a(idx):
                h, j = steps[idx]
                po = 64 * (h % 2)
                qwin = min(384, T - 128 * j)
                ps_s = ppool.tile([128, 384], f32, tag="ps",
                                  name=f"ps_s_{idx}")
                nc.tensor.matmul(
                    ps_s[:, :qwin],
                    lhsT=qkT_sb[po:po + 64, 2 + h // 2,
                                j * 128:(j + 1) * 128],
                    rhs=qkT_sb[po:po + 64, h // 2,
                               j * 128:j * 128 + qwin],
                    start=True,
                    stop=True,
                )
                pT = wpool.tile([128, 384], bf16, bufs=DELAY + 3,
                                name=f"pT_{idx}", tag="pT")
                nc.scalar.activation(pT[:, :qwin], ps_s[:, :qwin], AF.Exp)
                nc.vector.tensor_tensor(
                    out=pT[:, 0:128], in0=pT[:, 0:128],
                    in1=tris_sb[:, 0:128], op=OP.mult,
                )
                if qwin == 384:
                    nc.gpsimd.tensor_tensor(
                        out=pT[:, 256:384], in0=pT[:, 256:384],
                        in1=tris_sb[:, 128:256], op=OP.mult,
                    )
                pT_t[idx] = pT

            def stage_b(idx, pidx):
                h, j = steps[idx]
                po = 64 * (h % 2)
                qwin = min(384, T - 128 * j)
                pT = pT_t.pop(idx)
                for g in range((128 * j) // 512,
                               (128 * j + qwin - 1) // 512 + 1):
                    c0 = max(0, 512 * g - 128 * j)
                    c1 = min(qwin, 512 * (g + 1) - 128 * j)
                    if (h, g) not in ps_y:
                        ps_y[(h, g)] = ppool.tile(
                            [65, 512], f32, tag="ps", name=f"ps_y_{h}_{g}"
                        )
                    first = (j == max(0, 4 * g - 2))
                    last = (j == min(NT - 1, 4 * g + 3))
                    d0 = 128 * j + c0 - 512 * g
                    nc.tensor.matmul(
                        ps_y[(h, g)][:, d0:d0 + (c1 - c0)],
                        lhsT=vaug_sb[:, j, h, :],
                        rhs=pT[:, c0:c1],
                        start=first,
                        stop=last,
                        skip_group_check=True,
                    )
                    if not last:
                        continue
                    yps = ps_y.pop((h, g))
                    # reciprocal_approx_fast and partition_broadcast both
                    # require partition base 0: stage the denominator row
                    # at partition 0 first.
                    dn = wpool.tile([1, 512], f32, bufs=4,
                                    name=f"dn_{h}_{g}", tag="dn")
                    rec = wpool.tile([1, 512], f32, bufs=4,
                                     name=f"rec_{h}_{g}", tag="rec")
                    bc_sb = wpool.tile([64, 512], f32, bufs=3,
                                       name=f"bc_{h}_{g}", tag="bc")

                    def dn_copy(dn=dn, yps=yps):
                        nc.vector.tensor_copy(out=dn[:], in_=yps[64:65, :])

                    def do_recip(rec=rec, dn=dn):
                        nc.vector.reciprocal_approx_fast(rec[:], dn[:])

                    def do_bcast(bc_sb=bc_sb, rec=rec):
                        nc.gpsimd.partition_broadcast(bc_sb[:], rec[0:1, :])

                    def do_mult(yps=yps, bc_sb=bc_sb, po=po, h=h, g=g):
                        nc.vector.tensor_tensor(
                            out=yTn_sb[po:po + 64, h // 2,
                                       g * 512:(g + 1) * 512],
                            in0=yps[0:64, :],
                            in1=bc_sb[:],
                            op=OP.mult,
                        )

                    at_step(pidx + 1, dn_copy)
                    at_step(pidx + 2, do_recip)
                    at_step(pidx + 3, do_bcast)
                    at_step(pidx + 4, do_mult)
                    if h == HPC - 1:
                        at_step(pidx + 4, lambda g=g: out_proj(g))

            for idx in range(len(steps) + DELAY + 5):
                if idx < len(steps):
                    stage_a(idx)
                if DELAY <= idx < len(steps) + DELAY:
                    stage_b(idx - DELAY, idx)
                for fn in post.pop(idx, []):
                    fn()

    nc.compile()
    from concourse.bass_interp import get_hw_module

    nc.m = get_hw_module(nc.m)
    return nc


def _shard_inputs(x, Wqkv, bqkv, Wo, bo):
    import ml_dtypes

    bfdt = ml_dtypes.bfloat16

    x = np.asarray(x, np.float32)
    Wqkv = np.asarray(Wqkv, np.float32)
    bqkv = np.asarray(bqkv, np.float32)
    Wo = np.asarray(Wo, np.float32)

    scale = 1.0 / np.sqrt(np.float32(HD))
    c_idx = np.arange(128)[:, None]
    u_idx = np.arange(128)[None, :]
    tri0 = (u_idx >= c_idx).astype(np.float32)   # keys block j vs q block j
    tri1 = (u_idx < c_idx).astype(np.float32)    # keys block j vs q block j+2
    tris = np.concatenate([tri0, tri1], axis=1)

    in_maps = []
    for c in range(NCORES):
        b, hg = divmod(c, HPC)
        r0 = hg * FB
        Wq = Wqkv[r0:r0 + FB] * scale
        Wk = Wqkv[D + r0:D + r0 + FB]
        Wv = Wqkv[2 * D + r0:2 * D + r0 + FB]
        bq = bqkv[r0:r0 + FB] * scale
        bk = bqkv[D + r0:D + r0 + FB]
        bv = bqkv[2 * D + r0:2 * D + r0 + FB]
        in_maps.append({
            "xT": np.ascontiguousarray(x[b].T).astype(bfdt),
            "wqk": np.ascontiguousarray(
                np.concatenate([Wq, Wk], 0).T).astype(bfdt),
            "bqk": np.ascontiguousarray(
                np.concatenate([bq, bk]).reshape(4, 128).T),
            "wv": np.ascontiguousarray(Wv.T).astype(bfdt),
            "bvb": np.ascontiguousarray(
                np.broadcast_to(bv[None, :], (128, FB))
            ).reshape(128, HPC, HD),
            "wo": np.ascontiguousarray(Wo[:, r0:r0 + FB].T).astype(bfdt),
            "tris": tris.astype(bfdt),
            "vone": np.ones((128, 64), bfdt),
        })
    return in_maps


def kernel(x, Wqkv, bqkv, Wo, bo):
    from concourse import bass_utils

    if "nc" not in _STATE:
        _STATE["nc"] = _build_module()
    nc = _STATE["nc"]

    in_maps = _shard_inputs(x, Wqkv, bqkv, Wo, bo)
    trace = bool(os.environ.get("TRNKERN_TRACE"))
    res = bass_utils.run_bass_kernel_spmd(
        nc,
        in_maps,
        core_ids=list(range(NCORES)),
        trace=trace,
    )
    _STATE["last"] = res

    bo = np.asarray(bo, np.float32)
    out = np.empty((B, T, D), np.float32)
    for b in range(B):
        acc = res.results[b * HPC]["out_p"].astype(np.float32)
        for hg in range(1, HPC):
            acc = acc + res.results[b * HPC + hg]["out_p"].astype(np.float32)
        out[b] = acc + bo[None, :]
    return out


# revision 15
# speedup vs baseline: 2.6198x; 2.0969x over previous
"""Local (sliding-window) attention kernel for TRN2, 8 NeuronCores.

Sharding: core c -> batch b=c//4, head-group hg=c%4 (4 heads of 16).
Each core computes qkv projection for its heads, banded attention, and a
partial out-projection (its heads' columns of Wo). Host sums the 4
partials per batch and adds bo.

v2: all matmul operands bf16 (2x moving-operand stream rate vs fp32r,
keeps the PE HAM clock-gate warm), softmax normalization via
reciprocal_approx_fast + rank-1 ones-matmul partition broadcast
(replaces the 4us vector.reciprocal + 1.2us gpsimd broadcast serial
chain), bf16 output DMA (host upcasts + sums partials).

Device algorithm (per core):
  qkT[512,2048]  = wqk.T @ xT          (Q rows pre-scaled by 1/sqrt(hd))
  V  [2048,260]  = xT.T @ wv           (token-major; +bias, with a ones
                                        column per head -> vaug)
  per head h, key-block j (128 keys):
    S^T[k,q]     = kT_hj.T @ qT (q-window = 384 cols: blocks j..j+2)
    P^T          = exp(S^T) (bf16), zero band-complement triangles
    yT_psum[65,512] += vaug_hj.T @ P^T   (row 64 = softmax denominator)
  per (h, q-range g of 512):
    rec[1,512] = approx 1/denom (DVE custom op, from PSUM)
    bc[64,512] = ones[1,64].T @ rec      (PE rank-1 broadcast)
    yT (bf16)  = yT_psum * bc
  out[2048,1024] = yT.T @ wo  (partial; host adds across head-groups + bo)
"""

import os
import sys

import numpy as np

if "/opt/trn_rl_repo" not in sys.path:
    sys.path.insert(0, "/opt/trn_rl_repo")

B, T, D = 2, 2048, 1024
H, W = 16, 256
HD = D // H          # 64
NCORES = 8
HPC = 4              # heads per core
FB = HPC * HD        # 256 f-columns per core

_STATE: dict = {}


def _build_module():
    import concourse.bacc as bacc
    import concourse.tile as tile
    from concourse import mybir

    dt = mybir.dt
    AF = mybir.ActivationFunctionType
    OP = mybir.AluOpType

    nc = bacc.Bacc(
        "TRN2",
        target_bir_lowering=False,
        debug=False,
        enable_asserts=False,
        num_devices=NCORES,
    )

    f32 = dt.float32
    f32r = dt.float32r
    bf16 = dt.bfloat16
    xT_d = nc.dram_tensor("xT", [D, T], bf16, kind="ExternalInput").ap()
    wqk_d = nc.dram_tensor("wqk", [D, 2 * FB], bf16, kind="ExternalInput").ap()
    bqk_d = nc.dram_tensor("bqk", [128, 4], f32, kind="ExternalInput").ap()
    wv_d = nc.dram_tensor("wv", [D, FB], bf16, kind="ExternalInput").ap()
    bvb_d = nc.dram_tensor("bvb", [128, HPC, HD], f32, kind="ExternalInput").ap()
    wo_d = nc.dram_tensor("wo", [FB, D], bf16, kind="ExternalInput").ap()
    tris_d = nc.dram_tensor("tris", [128, 256], bf16, kind="ExternalInput").ap()
    vone_d = nc.dram_tensor("vone", [128, 64], bf16, kind="ExternalInput").ap()
    out_d = nc.dram_tensor("out_p", [T, D], bf16, kind="ExternalOutput").ap()

    KC = D // 128     # 8 contraction chunks
    NT = T // 128     # 16 token tiles / key blocks
    NQ = T // 512     # 4 q-ranges

    with tile.TileContext(nc) as tc:
        with (
            tc.tile_pool(name="const", bufs=1) as cpool,
            tc.tile_pool(name="work", bufs=3) as wpool,
            tc.tile_pool(name="ps", bufs=8, space="PSUM") as ppool,
        ):
            # ---- persistent SBUF ----
            xTa_sb = cpool.tile([128, KC, 1024], bf16)
            xTb_sb = cpool.tile([128, KC, 1024], bf16)
            wqk_sb = cpool.tile([128, KC, 2 * FB], bf16)
            wv_sb = cpool.tile([128, KC, FB], bf16)
            wo_sb = cpool.tile([128, 2, D], bf16)
            bqk_sb = cpool.tile([128, 4], f32)
            bvb_sb = cpool.tile([128, HPC, HD], f32)
            tris_sb = cpool.tile([128, 256], bf16)
            qkT_sb = cpool.tile([128, 4, T], bf16)
            vaug_sb = cpool.tile([128, NT, HPC, HD + 1], bf16)
            yTn_sb = cpool.tile([128, 2, T], bf16)

            # Two DMA queues (SP + ACT hwdge): group-0 operands (wqk, xTa)
            # lead both queues so the first matmul can start ~1us in;
            # constants follow, then second-half operands. ACT is idle
            # during the load phase.
            for a in range(KC):
                nc.sync.dma_start(wqk_sb[:, a, :], wqk_d[a * 128:(a + 1) * 128, :])
                nc.scalar.dma_start(
                    xTa_sb[:, a, :], xT_d[a * 128:(a + 1) * 128, 0:1024]
                )
            nc.sync.dma_start(bqk_sb[:], bqk_d[:])
            nc.sync.dma_start(bvb_sb[:], bvb_d[:])
            nc.sync.dma_start(tris_sb[:], tris_d[:])
            for h in range(HPC):
                nc.sync.dma_start(
                    vaug_sb[:, :, h, HD:HD + 1],
                    vone_d[:, h * 16:(h + 1) * 16, None],
                )
            for a in range(KC):
                nc.sync.dma_start(wv_sb[:, a, :], wv_d[a * 128:(a + 1) * 128, :])
            for a in range(KC):
                nc.scalar.dma_start(
                    xTb_sb[:, a, :], xT_d[a * 128:(a + 1) * 128, 1024:2048]
                )
            for f in range(2):
                nc.sync.dma_start(wo_sb[:, f, :], wo_d[f * 128:(f + 1) * 128, :])

            # ---- qkT projection: [512, 2048] ----
            # a-outer in groups of 8 PSUM tiles so the first pass streams
            # with the xT/wqk DMA arrivals instead of serializing one
            # accumulation chain against the whole load.
            def qkT_group(grp, xh_sb):
                tiles = [(m, n) for m in range(4) for n in range(2)]
                ps_g = {
                    mn: ppool.tile([128, 512], f32, tag="ps",
                                   name=f"ps_qk{grp}_{mn[0]}_{mn[1]}")
                    for mn in tiles
                }
                for a in range(KC):
                    for (m, n) in tiles:
                        nc.tensor.matmul(
                            ps_g[(m, n)][:],
                            lhsT=wqk_sb[:, a, m * 128:(m + 1) * 128],
                            rhs=xh_sb[:, a, n * 512:(n + 1) * 512],
                            start=(a == 0),
                            stop=(a == KC - 1),
                        )
                for (m, n) in tiles:
                    nc.scalar.activation(
                        qkT_sb[:, m, (2 * grp + n) * 512:(2 * grp + n + 1) * 512],
                        ps_g[(m, n)][:],
                        AF.Identity,
                        bias=bqk_sb[:, m:m + 1],
                    )

            def v_proj(t, xh_sb):
                tl = t % 8
                ps_v = ppool.tile([128, HPC, HD], f32, tag="ps", name=f"ps_v_{t}")
                for a in range(KC):
                    nc.tensor.matmul(
                        ps_v[:],
                        lhsT=xh_sb[:, a, tl * 128:(tl + 1) * 128],
                        rhs=wv_sb[:, a, :],
                        start=(a == 0),
                        stop=(a == KC - 1),
                    )
                for h in range(HPC):
                    nc.vector.tensor_tensor(
                        out=vaug_sb[:, t, h, 0:HD],
                        in0=ps_v[:, h, :],
                        in1=bvb_sb[:, h, :],
                        op=OP.add,
                    )

            qkT_group(0, xTa_sb)
            for t in range(8):
                v_proj(t, xTa_sb)
            qkT_group(1, xTb_sb)
            for t in range(8, NT):
                v_proj(t, xTb_sb)

            # ---- attention: software-pipelined over flattened (h, j) ----
            # stage A (step idx):   S^T matmul -> exp (ACT, PSUM -> bf16)
            #                       -> zero band-complement triangles
            # stage B (idx-DELAY):  pV matmuls
            # The (h,g)-tail normalize chain (denom copy -> approx recip ->
            # partition broadcast -> multiply) is staggered across LATER
            # python steps via `post`, so each op's inputs are already
            # complete when it reaches its engine's strict-FIFO head —
            # otherwise the chain head-blocks the DVE/GpSimd queues that
            # the per-step mask ops need, stalling the PE ~13us per group.
            DELAY = 4
            steps = [(h, j) for h in range(HPC) for j in range(NT)]
            pT_t = {}
            ps_y = {}
            post = {}

            def at_step(s, fn):
                post.setdefault(s, []).append(fn)

            def out_proj(g):
                for mt in range(4 * g, 4 * g + 4):
                    for nn in range(2):
                        ps_o = ppool.tile(
                            [128, 512], f32, tag="ps",
                            name=f"ps_o_{mt}_{nn}",
                        )
                        for fc in range(2):
                            nc.tensor.matmul(
                                ps_o[:],
                                lhsT=yTn_sb[:, fc, mt * 128:(mt + 1) * 128],
                                rhs=wo_sb[:, fc, nn * 512:(nn + 1) * 512],
                                start=(fc == 0),
                                stop=(fc == 1),
                            )
                        o_sb = wpool.tile(
                            [128, 512], bf16, bufs=4,
                            name=f"o_{mt}_{nn}", tag="o_sb",
                        )
                        if (mt + nn) % 2 == 0:
                            nc.vector.tensor_copy(out=o_sb[:], in_=ps_o[:])
                        else:
                            nc.scalar.copy(o_sb[:], ps_o[:])
                        nc.sync.dma_start(
                            out_d[mt * 128:(mt + 1) * 128,
                                  nn * 512:(nn + 1) * 512],
                            o_sb[:],
                        )

            def stage_a(idx):
                h, j = steps[idx]
                po = 64 * (h % 2)
                qwin = min(384, T - 128 * j)
                ps_s = ppool.tile([128, 384], f32, tag="ps",
                                  name=f"ps_s_{idx}")
                nc.tensor.matmul(
                    ps_s[:, :qwin],
                    lhsT=qkT_sb[po:po + 64, 2 + h // 2,
                                j * 128:(j + 1) * 128],
                    rhs=qkT_sb[po:po + 64, h // 2,
                               j * 128:j * 128 + qwin],
                    start=True,
                    stop=True,
                )
                pT = wpool.tile([128, 384], bf16, bufs=12,
                                name=f"pT_{idx}", tag="pT")
                nc.scalar.activation(pT[:, :qwin], ps_s[:, :qwin], AF.Exp)
                if qwin == 384:
                    # both band-complement triangles (cols 0:128 and
                    # 256:384) in one strided DVE op
                    pv = pT[:].rearrange("p (a b) -> p a b", a=3)[:, 0:3:2, :]
                    tv = tris_sb[:].rearrange("p (a b) -> p a b", a=2)
                    nc.vector.tensor_tensor(
                        out=pv, in0=pv, in1=tv, op=OP.mult,
                    )
                else:
                    nc.vector.tensor_tensor(
                        out=pT[:, 0:128], in0=pT[:, 0:128],
                        in1=tris_sb[:, 0:128], op=OP.mult,
                    )
                pT_t[idx] = pT

            def stage_b(idx, pidx):
                h, j = steps[idx]
                po = 64 * (h % 2)
                qwin = min(384, T - 128 * j)
                pT = pT_t.pop(idx)
                for g in range((128 * j) // 512,
                               (128 * j + qwin - 1) // 512 + 1):
                    c0 = max(0, 512 * g - 128 * j)
                    c1 = min(qwin, 512 * (g + 1) - 128 * j)
                    if (h, g) not in ps_y:
                        ps_y[(h, g)] = ppool.tile(
                            [65, 512], f32, tag="ps", name=f"ps_y_{h}_{g}"
                        )
                    first = (j == max(0, 4 * g - 2))
                    last = (j == min(NT - 1, 4 * g + 3))
                    d0 = 128 * j + c0 - 512 * g
                    nc.tensor.matmul(
                        ps_y[(h, g)][:, d0:d0 + (c1 - c0)],
                        lhsT=vaug_sb[:, j, h, :],
                        rhs=pT[:, c0:c1],
                        start=first,
                        stop=last,
                        skip_group_check=True,
                    )
                    if not last:
                        continue
                    yps = ps_y.pop((h, g))
                    # reciprocal_approx_fast and partition_broadcast both
                    # require partition base 0: stage the denominator row
                    # at partition 0 first.
                    dn = wpool.tile([1, 512], f32, bufs=4,
                                    name=f"dn_{h}_{g}", tag="dn")
                    rec = wpool.tile([1, 512], f32, bufs=4,
                                     name=f"rec_{h}_{g}", tag="rec")
                    bc_sb = wpool.tile([64, 512], f32, bufs=3,
                                       name=f"bc_{h}_{g}", tag="bc")

                    def dn_copy(dn=dn, yps=yps):
                        nc.scalar.copy(dn[:], yps[64:65, :])

                    def do_recip(rec=rec, dn=dn):
                        nc.vector.reciprocal_approx_fast(rec[:], dn[:])

                    def do_bcast(bc_sb=bc_sb, rec=rec):
                        nc.gpsimd.partition_broadcast(bc_sb[:], rec[0:1, :])

                    def do_mult(yps=yps, bc_sb=bc_sb, po=po, h=h, g=g):
                        nc.vector.tensor_tensor(
                            out=yTn_sb[po:po + 64, h // 2,
                                       g * 512:(g + 1) * 512],
                            in0=yps[0:64, :],
                            in1=bc_sb[:],
                            op=OP.mult,
                        )

                    at_step(pidx + 1, dn_copy)
                    at_step(pidx + 2, do_recip)
                    at_step(pidx + 3, do_bcast)
                    at_step(pidx + 4, do_mult)
                    if h == HPC - 1:
                        at_step(pidx + 4, lambda g=g: out_proj(g))

            for idx in range(len(steps) + DELAY + 5):
                if idx < len(steps):
                    stage_a(idx)
                if DELAY <= idx < len(steps) + DELAY:
                    stage_b(idx - DELAY, idx)
                for fn in post.pop(idx, []):
                    fn()

    nc.compile()
    from concourse.bass_interp import get_hw_module

    nc.m = get_hw_module(nc.m)
    return nc


def _shard_inputs(x, Wqkv, bqkv, Wo, bo):
    import ml_dtypes

    bfdt = ml_dtypes.bfloat16

    x = np.asarray(x, np.float32)
    Wqkv = np.asarray(Wqkv, np.float32)
    bqkv = np.asarray(bqkv, np.float32)
    Wo = np.asarray(Wo, np.float32)

    scale = 1.0 / np.sqrt(np.float32(HD))
    c_idx = np.arange(128)[:, None]
    u_idx = np.arange(128)[None, :]
    tri0 = (u_idx >= c_idx).astype(np.float32)   # keys block j vs q block j
    tri1 = (u_idx < c_idx).astype(np.float32)    # keys block j vs q block j+2
    tris = np.concatenate([tri0, tri1], axis=1)

    in_maps = []
    for c in range(NCORES):
        b, hg = divmod(c, HPC)
        r0 = hg * FB
        Wq = Wqkv[r0:r0 + FB] * scale
        Wk = Wqkv[D + r0:D + r0 + FB]
        Wv = Wqkv[2 * D + r0:2 * D + r0 + FB]
        bq = bqkv[r0:r0 + FB] * scale
        bk = bqkv[D + r0:D + r0 + FB]
        bv = bqkv[2 * D + r0:2 * D + r0 + FB]
        in_maps.append({
            "xT": np.ascontiguousarray(x[b].T).astype(bfdt),
            "wqk": np.ascontiguousarray(
                np.concatenate([Wq, Wk], 0).T).astype(bfdt),
            "bqk": np.ascontiguousarray(
                np.concatenate([bq, bk]).reshape(4, 128).T),
            "wv": np.ascontiguousarray(Wv.T).astype(bfdt),
            "bvb": np.ascontiguousarray(
                np.broadcast_to(bv[None, :], (128, FB))
            ).reshape(128, HPC, HD),
            "wo": np.ascontiguousarray(Wo[:, r0:r0 + FB].T).astype(bfdt),
            "tris": tris.astype(bfdt),
            "vone": np.ones((128, 64), bfdt),
        })
    return in_maps


def kernel(x, Wqkv, bqkv, Wo, bo):
    from concourse import bass_utils

    if "nc" not in _STATE:
        _STATE["nc"] = _build_module()
    nc = _STATE["nc"]

    in_maps = _shard_inputs(x, Wqkv, bqkv, Wo, bo)
    trace = bool(os.environ.get("TRNKERN_TRACE"))
    res = bass_utils.run_bass_kernel_spmd(
        nc,
        in_maps,
        core_ids=list(range(NCORES)),
        trace=trace,
    )
    _STATE["last"] = res

    bo = np.asarray(bo, np.float32)
    out = np.empty((B, T, D), np.float32)
    for b in range(B):
        acc = res.results[b * HPC]["out_p"].astype(np.float32)
        for hg in range(1, HPC):
            acc = acc + res.results[b * HPC + hg]["out_p"].astype(np.float32)
        out[b] = acc + bo[None, :]
    return out


# revision 18
# speedup vs baseline: 2.7343x; 1.0437x over previous
"""Local (sliding-window) attention kernel for TRN2, 8 NeuronCores.

Sharding: core c -> batch b=c//4, head-group hg=c%4 (4 heads of 16).
Each core computes qkv projection for its heads, banded attention, and a
partial out-projection (its heads' columns of Wo). Host sums the 4
partials per batch and adds bo.

v2: all matmul operands bf16 (2x moving-operand stream rate vs fp32r,
keeps the PE HAM clock-gate warm), softmax normalization via
reciprocal_approx_fast + rank-1 ones-matmul partition broadcast
(replaces the 4us vector.reciprocal + 1.2us gpsimd broadcast serial
chain), bf16 output DMA (host upcasts + sums partials).

Device algorithm (per core):
  qkT[512,2048]  = wqk.T @ xT          (Q rows pre-scaled by 1/sqrt(hd))
  V  [2048,260]  = xT.T @ wv           (token-major; +bias, with a ones
                                        column per head -> vaug)
  per head h, key-block j (128 keys):
    S^T[k,q]     = kT_hj.T @ qT (q-window = 384 cols: blocks j..j+2)
    P^T          = exp(S^T) (bf16), zero band-complement triangles
    yT_psum[65,512] += vaug_hj.T @ P^T   (row 64 = softmax denominator)
  per (h, q-range g of 512):
    rec[1,512] = approx 1/denom (DVE custom op, from PSUM)
    bc[64,512] = ones[1,64].T @ rec      (PE rank-1 broadcast)
    yT (bf16)  = yT_psum * bc
  out[2048,1024] = yT.T @ wo  (partial; host adds across head-groups + bo)
"""

import os
import sys

import numpy as np

if "/opt/trn_rl_repo" not in sys.path:
    sys.path.insert(0, "/opt/trn_rl_repo")

B, T, D = 2, 2048, 1024
H, W = 16, 256
HD = D // H          # 64
NCORES = 8
HPC = 4              # heads per core
FB = HPC * HD        # 256 f-columns per core

_STATE: dict = {}


def _build_module():
    import concourse.bacc as bacc
    import concourse.tile as tile
    from concourse import mybir

    dt = mybir.dt
    AF = mybir.ActivationFunctionType
    OP = mybir.AluOpType

    nc = bacc.Bacc(
        "TRN2",
        target_bir_lowering=False,
        debug=False,
        enable_asserts=False,
        num_devices=NCORES,
    )

    f32 = dt.float32
    f32r = dt.float32r
    bf16 = dt.bfloat16
    xT_d = nc.dram_tensor("xT", [D, T], bf16, kind="ExternalInput").ap()
    wqk_d = nc.dram_tensor("wqk", [D, 2 * FB], bf16, kind="ExternalInput").ap()
    bqk_d = nc.dram_tensor("bqk", [128, 4], f32, kind="ExternalInput").ap()
    wv_d = nc.dram_tensor("wv", [D, FB], bf16, kind="ExternalInput").ap()
    bvb_d = nc.dram_tensor("bvb", [128, HPC, HD], f32, kind="ExternalInput").ap()
    wo_d = nc.dram_tensor("wo", [FB, D], bf16, kind="ExternalInput").ap()
    tris_d = nc.dram_tensor("tris", [128, 256], bf16, kind="ExternalInput").ap()
    out_d = nc.dram_tensor("out_p", [T, D], bf16, kind="ExternalOutput").ap()

    KC = D // 128     # 8 contraction chunks
    NT = T // 128     # 16 token tiles / key blocks
    NQ = T // 512     # 4 q-ranges

    with tile.TileContext(nc) as tc:
        with (
            tc.tile_pool(name="const", bufs=1) as cpool,
            tc.tile_pool(name="work", bufs=3) as wpool,
            tc.tile_pool(name="ps", bufs=8, space="PSUM") as ppool,
        ):
            # ---- persistent SBUF ----
            # per-contraction-chunk tiles so each matmul depends only on
            # its own chunk's DMA, not the whole operand
            xTa_t = [cpool.tile([128, 1024], bf16, name=f"xTa{a}")
                     for a in range(KC)]
            xTb_t = [cpool.tile([128, 1024], bf16, name=f"xTb{a}")
                     for a in range(KC)]
            wqk_t = [cpool.tile([128, 2 * FB], bf16, name=f"wqk{a}")
                     for a in range(KC)]
            wv_t = [cpool.tile([128, FB], bf16, name=f"wv{a}")
                    for a in range(KC)]
            wo_sb = cpool.tile([128, 2, D], bf16)
            bqk_sb = cpool.tile([128, 4], f32)
            bvb_sb = cpool.tile([128, HPC, HD], f32)
            tris_sb = cpool.tile([128, 256], bf16)
            qkT_sb = cpool.tile([128, 4, T], bf16)
            vaug_sb = cpool.tile([128, NT, HPC, HD + 1], bf16)
            yTn_sb = cpool.tile([128, 2, T], bf16)

            # vaug ones columns via memset (DVE is idle during load)
            for h in range(HPC):
                nc.vector.memset(vaug_sb[:, :, h, HD:HD + 1], 1.0)

            # Two DMA queues (SP + ACT hwdge): group-0 operands (wqk, xTa)
            # lead both queues so the first matmul can start ~1us in;
            # constants follow, then second-half operands. ACT is idle
            # during the load phase.
            for a in range(KC):
                nc.sync.dma_start(wqk_t[a][:], wqk_d[a * 128:(a + 1) * 128, :])
                nc.scalar.dma_start(
                    xTa_t[a][:], xT_d[a * 128:(a + 1) * 128, 0:1024]
                )
            nc.sync.dma_start(bqk_sb[:], bqk_d[:])
            nc.sync.dma_start(bvb_sb[:], bvb_d[:])
            nc.sync.dma_start(tris_sb[:], tris_d[:])
            for a in range(KC):
                nc.sync.dma_start(wv_t[a][:], wv_d[a * 128:(a + 1) * 128, :])
            for a in range(KC):
                nc.scalar.dma_start(
                    xTb_t[a][:], xT_d[a * 128:(a + 1) * 128, 1024:2048]
                )
            for f in range(2):
                nc.sync.dma_start(wo_sb[:, f, :], wo_d[f * 128:(f + 1) * 128, :])

            # ---- qkT projection: [512, 2048] ----
            # a-outer in groups of 8 PSUM tiles so the first pass streams
            # with the xT/wqk DMA arrivals instead of serializing one
            # accumulation chain against the whole load.
            def qkT_group(grp, xh_t):
                tiles = [(m, n) for m in range(4) for n in range(2)]
                ps_g = {
                    mn: ppool.tile([128, 512], f32, tag="ps",
                                   name=f"ps_qk{grp}_{mn[0]}_{mn[1]}")
                    for mn in tiles
                }
                for a in range(KC):
                    for (m, n) in tiles:
                        nc.tensor.matmul(
                            ps_g[(m, n)][:],
                            lhsT=wqk_t[a][:, m * 128:(m + 1) * 128],
                            rhs=xh_t[a][:, n * 512:(n + 1) * 512],
                            start=(a == 0),
                            stop=(a == KC - 1),
                        )
                for (m, n) in tiles:
                    nc.scalar.activation(
                        qkT_sb[:, m, (2 * grp + n) * 512:(2 * grp + n + 1) * 512],
                        ps_g[(m, n)][:],
                        AF.Identity,
                        bias=bqk_sb[:, m:m + 1],
                    )

            def v_proj(t, xh_t):
                tl = t % 8
                ps_v = ppool.tile([128, HPC, HD], f32, tag="ps", name=f"ps_v_{t}")
                for a in range(KC):
                    nc.tensor.matmul(
                        ps_v[:],
                        lhsT=xh_t[a][:, tl * 128:(tl + 1) * 128],
                        rhs=wv_t[a][:],
                        start=(a == 0),
                        stop=(a == KC - 1),
                    )
                for h in range(HPC):
                    nc.vector.tensor_tensor(
                        out=vaug_sb[:, t, h, 0:HD],
                        in0=ps_v[:, h, :],
                        in1=bvb_sb[:, h, :],
                        op=OP.add,
                    )

            qkT_group(0, xTa_t)
            for t in range(8):
                v_proj(t, xTa_t)
            qkT_group(1, xTb_t)
            for t in range(8, NT):
                v_proj(t, xTb_t)

            # ---- attention: software-pipelined over flattened (h, j) ----
            # stage A (step idx):   S^T matmul -> exp (ACT, PSUM -> bf16)
            #                       -> zero band-complement triangles
            # stage B (idx-DELAY):  pV matmuls
            # The (h,g)-tail normalize chain (denom copy -> approx recip ->
            # partition broadcast -> multiply) is staggered across LATER
            # python steps via `post`, so each op's inputs are already
            # complete when it reaches its engine's strict-FIFO head —
            # otherwise the chain head-blocks the DVE/GpSimd queues that
            # the per-step mask ops need, stalling the PE ~13us per group.
            DELAY = 4
            steps = [(h, j) for h in range(HPC) for j in range(NT)]
            pT_t = {}
            ps_y = {}
            post = {}

            def at_step(s, fn):
                post.setdefault(s, []).append(fn)

            def out_proj(g):
                for mt in range(4 * g, 4 * g + 4):
                    for nn in range(2):
                        ps_o = ppool.tile(
                            [128, 512], f32, tag="ps",
                            name=f"ps_o_{mt}_{nn}",
                        )
                        for fc in range(2):
                            nc.tensor.matmul(
                                ps_o[:],
                                lhsT=yTn_sb[:, fc, mt * 128:(mt + 1) * 128],
                                rhs=wo_sb[:, fc, nn * 512:(nn + 1) * 512],
                                start=(fc == 0),
                                stop=(fc == 1),
                            )
                        o_sb = wpool.tile(
                            [128, 512], bf16, bufs=4,
                            name=f"o_{mt}_{nn}", tag="o_sb",
                        )
                        if (mt + nn) % 2 == 0:
                            nc.vector.tensor_copy(out=o_sb[:], in_=ps_o[:])
                        else:
                            nc.scalar.copy(o_sb[:], ps_o[:])
                        nc.sync.dma_start(
                            out_d[mt * 128:(mt + 1) * 128,
                                  nn * 512:(nn + 1) * 512],
                            o_sb[:],
                        )

            def stage_a(idx):
                h, j = steps[idx]
                po = 64 * (h % 2)
                qwin = min(384, T - 128 * j)
                ps_s = ppool.tile([128, 384], f32, tag="ps",
                                  name=f"ps_s_{idx}")
                nc.tensor.matmul(
                    ps_s[:, :qwin],
                    lhsT=qkT_sb[po:po + 64, 2 + h // 2,
                                j * 128:(j + 1) * 128],
                    rhs=qkT_sb[po:po + 64, h // 2,
                               j * 128:j * 128 + qwin],
                    start=True,
                    stop=True,
                )
                pT = wpool.tile([128, 384], bf16, bufs=12,
                                name=f"pT_{idx}", tag="pT")
                nc.scalar.activation(pT[:, :qwin], ps_s[:, :qwin], AF.Exp)
                if qwin == 384:
                    # both band-complement triangles (cols 0:128 and
                    # 256:384) in one strided DVE op
                    pv = pT[:].rearrange("p (a b) -> p a b", a=3)[:, 0:3:2, :]
                    tv = tris_sb[:].rearrange("p (a b) -> p a b", a=2)
                    nc.vector.tensor_tensor(
                        out=pv, in0=pv, in1=tv, op=OP.mult,
                    )
                else:
                    nc.vector.tensor_tensor(
                        out=pT[:, 0:128], in0=pT[:, 0:128],
                        in1=tris_sb[:, 0:128], op=OP.mult,
                    )
                pT_t[idx] = pT

            def stage_b(idx, pidx):
                h, j = steps[idx]
                po = 64 * (h % 2)
                qwin = min(384, T - 128 * j)
                pT = pT_t.pop(idx)
                for g in range((128 * j) // 512,
                               (128 * j + qwin - 1) // 512 + 1):
                    c0 = max(0, 512 * g - 128 * j)
                    c1 = min(qwin, 512 * (g + 1) - 128 * j)
                    if (h, g) not in ps_y:
                        ps_y[(h, g)] = ppool.tile(
                            [65, 512], f32, tag="ps", name=f"ps_y_{h}_{g}"
                        )
                    first = (j == max(0, 4 * g - 2))
                    last = (j == min(NT - 1, 4 * g + 3))
                    d0 = 128 * j + c0 - 512 * g
                    nc.tensor.matmul(
                        ps_y[(h, g)][:, d0:d0 + (c1 - c0)],
                        lhsT=vaug_sb[:, j, h, :],
                        rhs=pT[:, c0:c1],
                        start=first,
                        stop=last,
                        skip_group_check=True,
                    )
                    if not last:
                        continue
                    yps = ps_y.pop((h, g))
                    # reciprocal_approx_fast and partition_broadcast both
                    # require partition base 0: stage the denominator row
                    # at partition 0 first.
                    dn = wpool.tile([1, 512], f32, bufs=4,
                                    name=f"dn_{h}_{g}", tag="dn")
                    rec = wpool.tile([1, 512], f32, bufs=4,
                                     name=f"rec_{h}_{g}", tag="rec")
                    bc_sb = wpool.tile([64, 512], f32, bufs=3,
                                       name=f"bc_{h}_{g}", tag="bc")

                    def dn_copy(dn=dn, yps=yps):
                        nc.scalar.copy(dn[:], yps[64:65, :])

                    def do_recip(rec=rec, dn=dn):
                        nc.vector.reciprocal_approx_fast(rec[:], dn[:])

                    def do_bcast(bc_sb=bc_sb, rec=rec):
                        nc.gpsimd.partition_broadcast(bc_sb[:], rec[0:1, :])

                    def do_mult(yps=yps, bc_sb=bc_sb, po=po, h=h, g=g):
                        nc.vector.tensor_tensor(
                            out=yTn_sb[po:po + 64, h // 2,
                                       g * 512:(g + 1) * 512],
                            in0=yps[0:64, :],
                            in1=bc_sb[:],
                            op=OP.mult,
                        )

                    at_step(pidx + 1, dn_copy)
                    at_step(pidx + 2, do_recip)
                    at_step(pidx + 3, do_bcast)
                    at_step(pidx + 4, do_mult)
                    if h == HPC - 1:
                        at_step(pidx + 4, lambda g=g: out_proj(g))

            for idx in range(len(steps) + DELAY + 5):
                if idx < len(steps):
                    stage_a(idx)
                if DELAY <= idx < len(steps) + DELAY:
                    stage_b(idx - DELAY, idx)
                for fn in post.pop(idx, []):
                    fn()

    nc.compile()
    from concourse.bass_interp import get_hw_module

    nc.m = get_hw_module(nc.m)
    return nc


def _shard_inputs(x, Wqkv, bqkv, Wo, bo):
    import ml_dtypes

    bfdt = ml_dtypes.bfloat16

    x = np.asarray(x, np.float32)
    Wqkv = np.asarray(Wqkv, np.float32)
    bqkv = np.asarray(bqkv, np.float32)
    Wo = np.asarray(Wo, np.float32)

    scale = 1.0 / np.sqrt(np.float32(HD))
    c_idx = np.arange(128)[:, None]
    u_idx = np.arange(128)[None, :]
    tri0 = (u_idx >= c_idx).astype(np.float32)   # keys block j vs q block j
    tri1 = (u_idx < c_idx).astype(np.float32)    # keys block j vs q block j+2
    tris = np.concatenate([tri0, tri1], axis=1)

    in_maps = []
    for c in range(NCORES):
        b, hg = divmod(c, HPC)
        r0 = hg * FB
        Wq = Wqkv[r0:r0 + FB] * scale
        Wk = Wqkv[D + r0:D + r0 + FB]
        Wv = Wqkv[2 * D + r0:2 * D + r0 + FB]
        bq = bqkv[r0:r0 + FB] * scale
        bk = bqkv[D + r0:D + r0 + FB]
        bv = bqkv[2 * D + r0:2 * D + r0 + FB]
        in_maps.append({
            "xT": np.ascontiguousarray(x[b].T).astype(bfdt),
            "wqk": np.ascontiguousarray(
                np.concatenate([Wq, Wk], 0).T).astype(bfdt),
            "bqk": np.ascontiguousarray(
                np.concatenate([bq, bk]).reshape(4, 128).T),
            "wv": np.ascontiguousarray(Wv.T).astype(bfdt),
            "bvb": np.ascontiguousarray(
                np.broadcast_to(bv[None, :], (128, FB))
            ).reshape(128, HPC, HD),
            "wo": np.ascontiguousarray(Wo[:, r0:r0 + FB].T).astype(bfdt),
            "tris": tris.astype(bfdt),
            "vone": np.ones((128, 64), bfdt),
        })
    return in_maps


def kernel(x, Wqkv, bqkv, Wo, bo):
    from concourse import bass_utils

    if "nc" not in _STATE:
        _STATE["nc"] = _build_module()
    nc = _STATE["nc"]

    in_maps = _shard_inputs(x, Wqkv, bqkv, Wo, bo)
    trace = bool(os.environ.get("TRNKERN_TRACE"))
    res = bass_utils.run_bass_kernel_spmd(
        nc,
        in_maps,
        core_ids=list(range(NCORES)),
        trace=trace,
    )
    _STATE["last"] = res

    bo = np.asarray(bo, np.float32)
    out = np.empty((B, T, D), np.float32)
    for b in range(B):
        acc = res.results[b * HPC]["out_p"].astype(np.float32)
        for hg in range(1, HPC):
            acc = acc + res.results[b * HPC + hg]["out_p"].astype(np.float32)
        out[b] = acc + bo[None, :]
    return out
